# revision 18
# baseline (speedup 1.0000x reference)
"""2D DCT-II (separable) kernel for Trainium2, data-parallel over 8 NeuronCores.

Problem: img [128, 1, 512, 512] f32 -> out [128, 1, 512, 512] f32 with
    out[b,0] = scale * (Cp @ img[b,0] @ Cq^T)
where Cp[p,m] = cos(pi*(2m+1)*p/1024), Cq[q,n] = cos(pi*(2n+1)*q/1024) and
scale[p,q] = (2/512)*row[p]*col[q] (1/sqrt2 on p==0 / q==0). Since M=N=512 the
two basis matrices are identical; the rank-1 scale is folded into them:
    C'[k,j] = s_k * cos(pi*(2j+1)*k/1024),  s_k = sqrt(2/512) * (1/sqrt2 if k==0 else 1)
    out[b] = C' @ img[b] @ C'^T

Both stages run on the PE with the image/intermediate as the stationary
operand (the PE contracts over the partition dim, so no transposes needed):
    stage1: Dt[n, p] = sum_m A[m, n] * C'T[m, p]
    stage2: Y[p, q]  = sum_n Dt[n, p] * C'T[n, q]
Matmuls run in float32r (TF32-like, ~11-bit mantissa) at full PE rate.

Both stages exploit the DCT-II reflection symmetry C'[k, 511-j] = (-1)^k C'[k, j]:
with E[j'] = X[j'] + X[511-j'] and O[j'] = X[j'] - X[511-j'] (j' < 256), even
output indices need only a 256-long contraction with E, odd ones with O —
halving PE cycles per folded part.

Layout tricks that make the folds partition-aligned with plain forward APs:
 - the host passes the image as two row-halves (rows 0:256 forward, rows
   511:256 reversed) and column-permuted by pi = [0..255, 511..256];
 - stage-1 n-slices then give Dt tiles with n = [0:128, 128:256, 511:384desc,
   383:256desc], so stage-2 fold pairs (n, 511-n) are partition-aligned;
 - stage-1 writes even-p to PSUM cols 0:256 and odd-p to 256:512; the
   PSUM->SBUF copies interleave them (stride-2 writes, same 1x copy cost);
 - stage-2 even-q uses E2 = dt pairs folded on DVE/GpSimd; odd-q multiplies
   the raw dt slots against +-co (negated constant for the reversed slots),
   trading 2 extra PE matmuls for zero fold work.
"""

import sys
import numpy as np

for _p in ("/opt/trn_rl_repo", "/opt/pypackages"):
    if _p not in sys.path:
        sys.path.append(_p)

import concourse.tile as tile  # noqa: E402
from concourse import bacc, mybir  # noqa: E402
from concourse.bass_utils import run_bass_kernel_spmd  # noqa: E402

N_CORES = 8
B_FULL = 128
S = 512  # image side
H = S // 2
BPC = B_FULL // N_CORES  # images per core
T = S // 128  # 4 partition tiles per image side


def _basis_f32() -> np.ndarray:
    """C'T[j, k] = s_k * cos(pi*(2j+1)*k/1024), shape [512, 512] f32."""
    j = np.arange(S, dtype=np.float64)
    k = np.arange(S, dtype=np.float64)
    c = np.cos(np.pi * (2.0 * j[:, None] + 1.0) * k[None, :] / (2.0 * S))
    s = np.full(S, np.sqrt(2.0 / S), dtype=np.float64)
    s[0] /= np.sqrt(2.0)
    return (c * s[None, :]).astype(np.float32)


def _build():
    nc = bacc.Bacc("TRN2", target_bir_lowering=False, debug=False)
    imga_d = nc.dram_tensor(
        "imga", [BPC, H, S], mybir.dt.float32r, kind="ExternalInput"
    ).ap()
    imgb_d = nc.dram_tensor(
        "imgb", [BPC, H, S], mybir.dt.float32r, kind="ExternalInput"
    ).ap()
    ce_d = nc.dram_tensor("ce", [H, H], mybir.dt.float32r, kind="ExternalInput").ap()
    co_d = nc.dram_tensor("co", [H, H], mybir.dt.float32r, kind="ExternalInput").ap()
    conm_d = nc.dram_tensor("conm", [H, H], mybir.dt.float32r, kind="ExternalInput").ap()
    out_d = nc.dram_tensor("out", [BPC, S, S], mybir.dt.float32, kind="ExternalOutput").ap()

    out_v = out_d.rearrange("b (t p) q -> b p t q", p=128)
    imga_v = imga_d.rearrange("b (t p) n -> b p t n", p=128)
    imgb_v = imgb_d.rearrange("b (t p) n -> b p t n", p=128)
    ce_v = ce_d.rearrange("(t p) k -> t p k", p=128)
    co_v = co_d.rearrange("(t p) k -> t p k", p=128)
    conm_v = conm_d.rearrange("(t p) k -> t p k", p=128)

    with tile.TileContext(nc) as tc:
        with (
            tc.tile_pool(name="const", bufs=1) as cpool,
            tc.tile_pool(name="a", bufs=8) as apool,
            tc.tile_pool(name="eo", bufs=12) as eopool,
            tc.tile_pool(name="dt", bufs=2) as dtpool,
            tc.tile_pool(name="e2", bufs=4) as e2pool,
            tc.tile_pool(name="o", bufs=8) as opool,
            tc.tile_pool(name="ps1", bufs=4, space="PSUM") as ps1pool,
            tc.tile_pool(name="ps2", bufs=4, space="PSUM") as ps2pool,
        ):
            ce_sb = cpool.tile([128, 2, H], mybir.dt.float32r)
            co_sb = cpool.tile([128, 2, H], mybir.dt.float32r)
            conm_sb = cpool.tile([128, 2, H], mybir.dt.float32r)
            for t in range(2):
                nc.sync.dma_start(ce_sb[:, t, :], ce_v[t])
            for t in range(2):
                nc.sync.dma_start(co_sb[:, t, :], co_v[t])
            for t in range(2):
                nc.sync.dma_start(conm_sb[:, t, :], conm_v[t])

            for i in range(BPC):
                # Forward half and reversed half: partition j of (af[:,t],
                # ar[:,t]) is the fold pair (m', 511-m'). Columns are already
                # pi-permuted on the host.
                af = apool.tile([128, 2, S], mybir.dt.float32r, tag="a", name=f"af_{i}")
                ar = apool.tile([128, 2, S], mybir.dt.float32r, tag="a", name=f"ar_{i}")
                nc.sync.dma_start(af[:], imga_v[i])
                nc.sync.dma_start(ar[:], imgb_v[i])

                # Row fold: E = A + rev(A) (DVE/GpSimd), O = A - rev(A) (GpSimd)
                e0 = eopool.tile([128, S], mybir.dt.float32r, tag="eo", name=f"e0_{i}")
                e1 = eopool.tile([128, S], mybir.dt.float32r, tag="eo", name=f"e1_{i}")
                o0 = eopool.tile([128, S], mybir.dt.float32r, tag="eo", name=f"o0_{i}")
                o1 = eopool.tile([128, S], mybir.dt.float32r, tag="eo", name=f"o1_{i}")
                nc.vector.tensor_add(e0[:], af[:, 0, :], ar[:, 0, :])
                nc.gpsimd.tensor_add(e1[:], af[:, 1, :], ar[:, 1, :])
                nc.gpsimd.tensor_sub(o0[:], af[:, 0, :], ar[:, 0, :])
                nc.gpsimd.tensor_sub(o1[:], af[:, 1, :], ar[:, 1, :])
                e_t, o_t = (e0, e1), (o0, o1)

                # stage 1 (folded): ps1[nt] cols 0:256 even-p (E/ce), cols
                # 256:512 odd-p (O/co). n-slice nt covers pi-permuted columns,
                # i.e. n = [0:128, 128:256, 511:384desc, 383:256desc][nt].
                ps1 = [ps1pool.tile([128, S], mybir.dt.float32, tag="ps1", name=f"ps1_{i}_{j}") for j in range(T)]
                for nt in range(T):
                    nts = slice(nt * 128, (nt + 1) * 128)
                    for mh in range(2):
                        nc.tensor.matmul(
                            ps1[nt][:, 0:H],
                            e_t[mh][:, nts],
                            ce_sb[:, mh, :],
                            start=(mh == 0),
                            stop=(mh == 1),
                        )
                    for mh in range(2):
                        nc.tensor.matmul(
                            ps1[nt][:, H:S],
                            o_t[mh][:, nts],
                            co_sb[:, mh, :],
                            start=(mh == 0),
                            stop=(mh == 1),
                        )
                # Mid copies interleave even/odd p (stride-2 writes). Split
                # DVE / ACT to balance engine load.
                dt_sb = dtpool.tile([128, T, S], mybir.dt.float32r, tag="dt")
                for nt in range(T):
                    eng = nc.vector.tensor_copy if nt < 2 else nc.scalar.copy
                    eng(dt_sb[:, nt, 0:S:2], ps1[nt][:, 0:H])
                    eng(dt_sb[:, nt, 1:S:2], ps1[nt][:, H:S])

                # Column fold for stage-2 even-q: E2 pairs (n, 511-n) are
                # partition-aligned thanks to the pi layout: slots (0,2), (1,3).
                e2a = e2pool.tile([128, S], mybir.dt.float32r, tag="e2", name=f"e2a_{i}")
                e2b = e2pool.tile([128, S], mybir.dt.float32r, tag="e2", name=f"e2b_{i}")
                nc.vector.tensor_add(e2a[:], dt_sb[:, 0, :], dt_sb[:, 2, :])
                nc.vector.tensor_add(e2b[:], dt_sb[:, 1, :], dt_sb[:, 3, :])

                # stage 2: per p-tile, even-q = E2 x ce (2 matmuls); odd-q =
                # raw dt slots x [co, co, -co, -co] (4 matmuls — the reversed
                # slots 2,3 flip sign under the odd symmetry).
                for ph in range(2):
                    o_sb = opool.tile(
                        [128, 2, S], mybir.dt.float32, tag="o", name=f"o_{i}_{ph}"
                    )
                    for pj in range(2):
                        pt = ph * 2 + pj
                        pts = slice(pt * 128, (pt + 1) * 128)
                        ps2 = ps2pool.tile(
                            [128, S], mybir.dt.float32, tag="ps2", name=f"ps2_{i}_{pt}"
                        )
                        nc.tensor.matmul(ps2[:, 0:H], e2a[:, pts], ce_sb[:, 0, :], start=True, stop=False)
                        nc.tensor.matmul(ps2[:, 0:H], e2b[:, pts], ce_sb[:, 1, :], start=False, stop=True)
                        nc.tensor.matmul(ps2[:, H:S], dt_sb[:, 0, pts], co_sb[:, 0, :], start=True, stop=False)
                        nc.tensor.matmul(ps2[:, H:S], dt_sb[:, 1, pts], co_sb[:, 1, :], start=False, stop=False)
                        nc.tensor.matmul(ps2[:, H:S], dt_sb[:, 2, pts], conm_sb[:, 0, :], start=False, stop=False)
                        nc.tensor.matmul(ps2[:, H:S], dt_sb[:, 3, pts], conm_sb[:, 1, :], start=False, stop=True)
                        # Final interleave: even q from cols 0:256, odd from 256:512.
                        eng = nc.scalar.copy if pj == 0 else nc.vector.tensor_copy
                        eng(o_sb[:, pj, 0:S:2], ps2[:, 0:H])
                        eng(o_sb[:, pj, 1:S:2], ps2[:, H:S])
                    if ph == 0:
                        nc.scalar.dma_start(out_v[i, :, 0:2, :], o_sb[:])
                    else:
                        nc.sync.dma_start(out_v[i, :, 2:4, :], o_sb[:])
    nc.compile()
    return nc


_NC_CACHE = None


def _get_nc():
    global _NC_CACHE
    if _NC_CACHE is None:
        _NC_CACHE = _build()
    return _NC_CACHE


def run_sharded(img: np.ndarray, **spmd_kwargs):
    """img [128, 1, 512, 512] f32 -> (out [128, 1, 512, 512] f32, BassKernelResults)."""
    img = np.asarray(img, dtype=np.float32).reshape(B_FULL, S, S)
    perm = np.concatenate([np.arange(H), np.arange(S - 1, H - 1, -1)])
    imgp = img[:, :, perm]
    imga = np.ascontiguousarray(imgp[:, :H, :])
    imgb = np.ascontiguousarray(imgp[:, : H - 1 : -1, :])  # rows 511..256 reversed
    ct = _basis_f32()
    ce = np.ascontiguousarray(ct[:H, 0::2])
    co = np.ascontiguousarray(ct[:H, 1::2])
    conm = np.ascontiguousarray(-co)
    nc = _get_nc()
    in_maps = [
        {
            "imga": imga[k * BPC : (k + 1) * BPC],
            "imgb": imgb[k * BPC : (k + 1) * BPC],
            "ce": ce,
            "co": co,
            "conm": conm,
        }
        for k in range(N_CORES)
    ]
    res = run_bass_kernel_spmd(nc, in_maps, core_ids=list(range(N_CORES)), **spmd_kwargs)
    out = np.empty((B_FULL, S, S), dtype=np.float32)
    for k in range(N_CORES):
        out[k * BPC : (k + 1) * BPC] = res.results[k]["out"]
    return out.reshape(B_FULL, 1, S, S), res


def kernel(img: np.ndarray) -> np.ndarray:
    out, _ = run_sharded(img)
    return out


# revision 23
# speedup vs baseline: 1.1207x; 1.1207x over previous
"""2D DCT-II (separable) kernel for Trainium2, data-parallel over 8 NeuronCores.

Problem: img [128, 1, 512, 512] f32 -> out [128, 1, 512, 512] f32 with
    out[b,0] = scale * (Cp @ img[b,0] @ Cq^T)
where Cp[p,m] = cos(pi*(2m+1)*p/1024), Cq[q,n] = cos(pi*(2n+1)*q/1024) and
scale[p,q] = (2/512)*row[p]*col[q] (1/sqrt2 on p==0 / q==0). Since M=N=512 the
two basis matrices are identical; the rank-1 scale is folded into them:
    C'[k,j] = s_k * cos(pi*(2j+1)*k/1024),  s_k = sqrt(2/512) * (1/sqrt2 if k==0 else 1)
    out[b] = C' @ img[b] @ C'^T

Per-core (16 images each): two PE matmul stages with the image/intermediate as
the stationary operand (both stages contract over the data's partition dim, so
no transposes are needed):
    stage1: Dt[n, p] = sum_m A[m, n] * C'T[m, p]   (lhsT = A tile, rhs = C'T)
    stage2: Y[p, q]  = sum_n Dt[n, p] * C'T[n, q]  (lhsT = Dt tile, rhs = C'T)
Matmuls run in float32r (TF32-like, ~11 mantissa bits) at full PE rate.

Stage 1 exploits the DCT-II reflection symmetry C'[p, 511-m] = (-1)^p C'[p, m]:
with E[m'] = A[m'] + A[511-m'] and O[m'] = A[m'] - A[511-m'] (m' < 256),
even output rows come from a 256-contraction with E, odd rows from O —
half the stage-1 PE cycles. The reversed operand rows are loaded by DMA
(negative row stride); E/O are formed on DVE/GpSimd; the even/odd column
interleave happens inside the PSUM->SBUF copies (stride-2 writes, same cost).
"""

import sys
import numpy as np

for _p in ("/opt/trn_rl_repo", "/opt/pypackages"):
    if _p not in sys.path:
        sys.path.append(_p)

import concourse.tile as tile  # noqa: E402
from concourse import bacc, mybir  # noqa: E402
from concourse.bass_utils import run_bass_kernel_spmd  # noqa: E402

N_CORES = 8
B_FULL = 128
S = 512  # image side
H = S // 2
BPC = B_FULL // N_CORES  # images per core
T = S // 128  # 4 partition tiles per image side


def _basis_f32() -> np.ndarray:
    """C'T[j, k] = s_k * cos(pi*(2j+1)*k/1024), shape [512, 512] f32."""
    j = np.arange(S, dtype=np.float64)
    k = np.arange(S, dtype=np.float64)
    c = np.cos(np.pi * (2.0 * j[:, None] + 1.0) * k[None, :] / (2.0 * S))
    s = np.full(S, np.sqrt(2.0 / S), dtype=np.float64)
    s[0] /= np.sqrt(2.0)
    return (c * s[None, :]).astype(np.float32)


def _build():
    nc = bacc.Bacc("TRN2", target_bir_lowering=False, debug=False)
    # Image passed as two halves: rows 0:256 forward, rows 511:255 reversed
    # (host-side flip) so fold pairs m' <-> 511-m' are partition-aligned with
    # plain positive-stride DMAs.
    imga_d = nc.dram_tensor(
        "imga", [BPC, H, S], mybir.dt.float32r, kind="ExternalInput"
    ).ap()
    imgb_d = nc.dram_tensor(
        "imgb", [BPC, H, S], mybir.dt.float32r, kind="ExternalInput"
    ).ap()
    ct_d = nc.dram_tensor("ct", [S, S], mybir.dt.float32r, kind="ExternalInput").ap()
    ce_d = nc.dram_tensor("ce", [H, H], mybir.dt.float32r, kind="ExternalInput").ap()
    co_d = nc.dram_tensor("co", [H, H], mybir.dt.float32r, kind="ExternalInput").ap()
    out_d = nc.dram_tensor("out", [BPC, S, S], mybir.dt.float32, kind="ExternalOutput").ap()

    out_v = out_d.rearrange("b (t p) q -> b p t q", p=128)
    imga_v = imga_d.rearrange("b (t p) n -> b p t n", p=128)
    imgb_v = imgb_d.rearrange("b (t p) n -> b p t n", p=128)
    ct_v = ct_d.rearrange("(t p) k -> t p k", p=128)
    ce_v = ce_d.rearrange("(t p) k -> t p k", p=128)
    co_v = co_d.rearrange("(t p) k -> t p k", p=128)

    with tile.TileContext(nc) as tc:
        with (
            tc.tile_pool(name="const", bufs=1) as cpool,
            tc.tile_pool(name="a", bufs=10) as apool,
            tc.tile_pool(name="eo", bufs=16) as eopool,
            tc.tile_pool(name="dt", bufs=2) as dtpool,
            tc.tile_pool(name="o", bufs=8) as opool,
            tc.tile_pool(name="ps1", bufs=4, space="PSUM") as ps1pool,
            tc.tile_pool(name="ps2", bufs=4, space="PSUM") as ps2pool,
        ):
            # ce tile 0 first — the very first matmul needs only it plus
            # image 0's two halves; the remaining constants follow them.
            ce_sb = cpool.tile([128, 2, H], mybir.dt.float32r)
            co_sb = cpool.tile([128, 2, H], mybir.dt.float32r)
            ct_sb = cpool.tile([128, T, S], mybir.dt.float32r)
            nc.sync.dma_start(ce_sb[:, 0, :], ce_v[0])

            for i in range(BPC):
                # Forward half [m' tile, n] and reversed half: ar[:, t, :]
                # holds rows 511-... so partition j of (af[:,t], ar[:,t]) is
                # the fold pair (m', 511-m').
                af = apool.tile([128, 2, S], mybir.dt.float32r, tag="a", name=f"af_{i}")
                ar = apool.tile([128, 2, S], mybir.dt.float32r, tag="a", name=f"ar_{i}")
                nc.sync.dma_start(af[:], imga_v[i])
                nc.sync.dma_start(ar[:], imgb_v[i])
                if i == 0:
                    # Remaining constants, ordered by first use.
                    nc.sync.dma_start(ce_sb[:, 1, :], ce_v[1])
                    for t in range(2):
                        nc.sync.dma_start(co_sb[:, t, :], co_v[t])
                    for t in range(T):
                        nc.sync.dma_start(ct_sb[:, t, :], ct_v[t])

                # Fold: E = A + rev(A), O = A - rev(A)  (each [256, 512], 2 tiles)
                e0 = eopool.tile([128, S], mybir.dt.float32r, tag="eo", name=f"e0_{i}")
                e1 = eopool.tile([128, S], mybir.dt.float32r, tag="eo", name=f"e1_{i}")
                o0 = eopool.tile([128, S], mybir.dt.float32r, tag="eo", name=f"o0_{i}")
                o1 = eopool.tile([128, S], mybir.dt.float32r, tag="eo", name=f"o1_{i}")
                nc.vector.tensor_add(e0[:], af[:, 0, :], ar[:, 0, :])
                nc.vector.tensor_add(e1[:], af[:, 1, :], ar[:, 1, :])
                nc.gpsimd.tensor_sub(o0[:], af[:, 0, :], ar[:, 0, :])
                nc.gpsimd.tensor_sub(o1[:], af[:, 1, :], ar[:, 1, :])
                e_t, o_t = (e0, e1), (o0, o1)

                # stage 1 (folded): Dt[n, 2k] from E/ce, Dt[n, 2k+1] from O/co.
                # ps1[nt] cols 0:256 hold even-p, cols 256:512 odd-p.
                ps1 = [ps1pool.tile([128, S], mybir.dt.float32, tag="ps1", name=f"ps1_{i}_{j}") for j in range(T)]
                for nt in range(T):
                    nts = slice(nt * 128, (nt + 1) * 128)
                    for mh in range(2):
                        nc.tensor.matmul(
                            ps1[nt][:, 0:H],
                            e_t[mh][:, nts],
                            ce_sb[:, mh, :],
                            start=(mh == 0),
                            stop=(mh == 1),
                        )
                    for mh in range(2):
                        nc.tensor.matmul(
                            ps1[nt][:, H:S],
                            o_t[mh][:, nts],
                            co_sb[:, mh, :],
                            start=(mh == 0),
                            stop=(mh == 1),
                        )
                dt_sb = dtpool.tile([128, T, S], mybir.dt.float32r, tag="dt")
                for nt in range(T):
                    # One mid-copy pair on ACT to keep DVE under the PE span.
                    eng = nc.scalar.copy if nt == 3 else nc.vector.tensor_copy
                    eng(dt_sb[:, nt, 0:S:2], ps1[nt][:, 0:H])
                    eng(dt_sb[:, nt, 1:S:2], ps1[nt][:, H:S])

                # stage 2 (p-outer): Y[p, q] = sum_n Dt[n, p] C'T[n, q]
                # Output staged in 2-tile chunks: fewer DMA descriptors while
                # keeping the drain pipelined.
                for ph in range(2):
                    o_sb = opool.tile(
                        [128, 2, S], mybir.dt.float32, tag="o", name=f"o_{i}_{ph}"
                    )
                    for pj in range(2):
                        pt = ph * 2 + pj
                        ps2 = ps2pool.tile(
                            [128, S], mybir.dt.float32, tag="ps2", name=f"ps2_{i}_{pt}"
                        )
                        for nt in range(T):
                            nc.tensor.matmul(
                                ps2[:],
                                dt_sb[:, nt, pt * 128 : (pt + 1) * 128],
                                ct_sb[:, nt, :],
                                start=(nt == 0),
                                stop=(nt == T - 1),
                            )
                        nc.scalar.copy(o_sb[:, pj, :], ps2[:])
                    if ph == 0:
                        nc.scalar.dma_start(out_v[i, :, 0:2, :], o_sb[:])
                    else:
                        nc.sync.dma_start(out_v[i, :, 2:4, :], o_sb[:])
    nc.compile()
    return nc


_NC_CACHE = None


def _get_nc():
    global _NC_CACHE
    if _NC_CACHE is None:
        _NC_CACHE = _build()
    return _NC_CACHE


def run_sharded(img: np.ndarray, **spmd_kwargs):
    """img [128, 1, 512, 512] f32 -> (out [128, 1, 512, 512] f32, BassKernelResults)."""
    img = np.ascontiguousarray(np.asarray(img, dtype=np.float32)).reshape(B_FULL, S, S)
    imga = np.ascontiguousarray(img[:, :H, :])
    imgb = np.ascontiguousarray(img[:, :H - 1 :-1, :])  # rows 511..256 reversed
    ct = _basis_f32()
    ce = np.ascontiguousarray(ct[:H, 0::2])
    co = np.ascontiguousarray(ct[:H, 1::2])
    nc = _get_nc()
    in_maps = [
        {
            "imga": imga[k * BPC : (k + 1) * BPC],
            "imgb": imgb[k * BPC : (k + 1) * BPC],
            "ct": ct,
            "ce": ce,
            "co": co,
        }
        for k in range(N_CORES)
    ]
    res = run_bass_kernel_spmd(nc, in_maps, core_ids=list(range(N_CORES)), **spmd_kwargs)
    out = np.empty((B_FULL, S, S), dtype=np.float32)
    for k in range(N_CORES):
        out[k * BPC : (k + 1) * BPC] = res.results[k]["out"]
    return out.reshape(B_FULL, 1, S, S), res


def kernel(img: np.ndarray) -> np.ndarray:
    out, _ = run_sharded(img)
    return out


# revision 27
# speedup vs baseline: 1.1243x; 1.0033x over previous
"""2D DCT-II (separable) kernel for Trainium2, data-parallel over 8 NeuronCores.

Problem: img [128, 1, 512, 512] f32 -> out [128, 1, 512, 512] f32 with
    out[b,0] = scale * (Cp @ img[b,0] @ Cq^T)
where Cp[p,m] = cos(pi*(2m+1)*p/1024), Cq[q,n] = cos(pi*(2n+1)*q/1024) and
scale[p,q] = (2/512)*row[p]*col[q] (1/sqrt2 on p==0 / q==0). Since M=N=512 the
two basis matrices are identical; the rank-1 scale is folded into them:
    C'[k,j] = s_k * cos(pi*(2j+1)*k/1024),  s_k = sqrt(2/512) * (1/sqrt2 if k==0 else 1)
    out[b] = C' @ img[b] @ C'^T

Per-core (16 images each): two PE matmul stages with the image/intermediate as
the stationary operand (both stages contract over the data's partition dim, so
no transposes are needed):
    stage1: Dt[n, p] = sum_m A[m, n] * C'T[m, p]   (lhsT = A tile, rhs = C'T)
    stage2: Y[p, q]  = sum_n Dt[n, p] * C'T[n, q]  (lhsT = Dt tile, rhs = C'T)
Matmuls run in float32r (TF32-like, ~11 mantissa bits) at full PE rate.

Stage 1 exploits the DCT-II reflection symmetry C'[p, 511-m] = (-1)^p C'[p, m]:
with E[m'] = A[m'] + A[511-m'] and O[m'] = A[m'] - A[511-m'] (m' < 256),
even output rows come from a 256-contraction with E, odd rows from O —
half the stage-1 PE cycles. The reversed operand rows are loaded by DMA
(negative row stride); E/O are formed on DVE/GpSimd; the even/odd column
interleave happens inside the PSUM->SBUF copies (stride-2 writes, same cost).
"""

import sys
import numpy as np

for _p in ("/opt/trn_rl_repo", "/opt/pypackages"):
    if _p not in sys.path:
        sys.path.append(_p)

import concourse.tile as tile  # noqa: E402
from concourse import bacc, mybir  # noqa: E402
from concourse.bass_utils import run_bass_kernel_spmd  # noqa: E402

N_CORES = 8
B_FULL = 128
S = 512  # image side
H = S // 2
BPC = B_FULL // N_CORES  # images per core
T = S // 128  # 4 partition tiles per image side


def _basis_f32() -> np.ndarray:
    """C'T[j, k] = s_k * cos(pi*(2j+1)*k/1024), shape [512, 512] f32."""
    j = np.arange(S, dtype=np.float64)
    k = np.arange(S, dtype=np.float64)
    c = np.cos(np.pi * (2.0 * j[:, None] + 1.0) * k[None, :] / (2.0 * S))
    s = np.full(S, np.sqrt(2.0 / S), dtype=np.float64)
    s[0] /= np.sqrt(2.0)
    return (c * s[None, :]).astype(np.float32)


def _build():
    nc = bacc.Bacc("TRN2", target_bir_lowering=False, debug=False)
    # Image passed as two halves: rows 0:256 forward, rows 511:255 reversed
    # (host-side flip) so fold pairs m' <-> 511-m' are partition-aligned with
    # plain positive-stride DMAs.
    imga_d = nc.dram_tensor(
        "imga", [BPC, H, S], mybir.dt.float32r, kind="ExternalInput"
    ).ap()
    imgb_d = nc.dram_tensor(
        "imgb", [BPC, H, S], mybir.dt.float32r, kind="ExternalInput"
    ).ap()
    ct_d = nc.dram_tensor("ct", [S, S], mybir.dt.float32r, kind="ExternalInput").ap()
    ce_d = nc.dram_tensor("ce", [H, H], mybir.dt.float32r, kind="ExternalInput").ap()
    co_d = nc.dram_tensor("co", [H, H], mybir.dt.float32r, kind="ExternalInput").ap()
    out_d = nc.dram_tensor("out", [BPC, S, S], mybir.dt.float32, kind="ExternalOutput").ap()

    out_v = out_d.rearrange("b (t p) q -> b p t q", p=128)
    imga_v = imga_d.rearrange("b (t p) n -> b p t n", p=128)
    imgb_v = imgb_d.rearrange("b (t p) n -> b p t n", p=128)
    ct_v = ct_d.rearrange("(t p) k -> t p k", p=128)
    ce_v = ce_d.rearrange("(t p) k -> t p k", p=128)
    co_v = co_d.rearrange("(t p) k -> t p k", p=128)

    with tile.TileContext(nc) as tc:
        with (
            tc.tile_pool(name="const", bufs=1) as cpool,
            tc.tile_pool(name="a", bufs=10) as apool,
            tc.tile_pool(name="eo", bufs=16) as eopool,
            tc.tile_pool(name="dt", bufs=2) as dtpool,
            tc.tile_pool(name="o", bufs=8) as opool,
            tc.tile_pool(name="ps1", bufs=4, space="PSUM") as ps1pool,
            tc.tile_pool(name="ps2", bufs=4, space="PSUM") as ps2pool,
        ):
            # ce tile 0 first — the very first matmul needs only it plus
            # image 0's two halves; the remaining constants follow them.
            ce_sb = cpool.tile([128, 2, H], mybir.dt.float32r)
            co_sb = cpool.tile([128, 2, H], mybir.dt.float32r)
            ct_sb = cpool.tile([128, T, S], mybir.dt.float32r)
            nc.sync.dma_start(co_sb[:, 0, :], co_v[0])

            for i in range(BPC):
                # Forward half [m' tile, n] and reversed half: ar[:, t, :]
                # holds rows 511-... so partition j of (af[:,t], ar[:,t]) is
                # the fold pair (m', 511-m').
                af = apool.tile([128, 2, S], mybir.dt.float32r, tag="a", name=f"af_{i}")
                ar = apool.tile([128, 2, S], mybir.dt.float32r, tag="a", name=f"ar_{i}")
                nc.sync.dma_start(af[:], imga_v[i])
                nc.sync.dma_start(ar[:], imgb_v[i])
                if i == 0:
                    # Remaining constants, ordered by first use.
                    nc.sync.dma_start(co_sb[:, 1, :], co_v[1])
                    for t in range(2):
                        nc.sync.dma_start(ce_sb[:, t, :], ce_v[t])
                    for t in range(T):
                        nc.sync.dma_start(ct_sb[:, t, :], ct_v[t])

                # Fold: E = A + rev(A), O = A - rev(A)  (each [256, 512], 2 tiles)
                e0 = eopool.tile([128, S], mybir.dt.float32r, tag="eo", name=f"e0_{i}")
                e1 = eopool.tile([128, S], mybir.dt.float32r, tag="eo", name=f"e1_{i}")
                o0 = eopool.tile([128, S], mybir.dt.float32r, tag="eo", name=f"o0_{i}")
                o1 = eopool.tile([128, S], mybir.dt.float32r, tag="eo", name=f"o1_{i}")
                nc.gpsimd.tensor_sub(o0[:], af[:, 0, :], ar[:, 0, :])
                nc.gpsimd.tensor_sub(o1[:], af[:, 1, :], ar[:, 1, :])
                nc.vector.tensor_add(e0[:], af[:, 0, :], ar[:, 0, :])
                nc.vector.tensor_add(e1[:], af[:, 1, :], ar[:, 1, :])
                e_t, o_t = (e0, e1), (o0, o1)

                # stage 1 (folded): Dt[n, 2k] from E/ce, Dt[n, 2k+1] from O/co.
                # ps1[nt] cols 0:256 hold even-p, cols 256:512 odd-p.
                ps1 = [ps1pool.tile([128, S], mybir.dt.float32, tag="ps1", name=f"ps1_{i}_{j}") for j in range(T)]
                # O-part first: the gpsimd subs are ready earlier than the DVE
                # adds (which queue behind the previous image's mid copies).
                for nt in range(T):
                    nts = slice(nt * 128, (nt + 1) * 128)
                    for mh in range(2):
                        nc.tensor.matmul(
                            ps1[nt][:, H:S],
                            o_t[mh][:, nts],
                            co_sb[:, mh, :],
                            start=(mh == 0),
                            stop=(mh == 1),
                        )
                    for mh in range(2):
                        nc.tensor.matmul(
                            ps1[nt][:, 0:H],
                            e_t[mh][:, nts],
                            ce_sb[:, mh, :],
                            start=(mh == 0),
                            stop=(mh == 1),
                        )
                dt_sb = dtpool.tile([128, T, S], mybir.dt.float32r, tag="dt")
                for nt in range(T):
                    # One mid-copy pair on ACT to keep DVE under the PE span.
                    eng = nc.scalar.copy if nt == 3 else nc.vector.tensor_copy
                    eng(dt_sb[:, nt, 0:S:2], ps1[nt][:, 0:H])
                    eng(dt_sb[:, nt, 1:S:2], ps1[nt][:, H:S])

                # stage 2 (p-outer): Y[p, q] = sum_n Dt[n, p] C'T[n, q]
                # Output staged in 2-tile chunks: fewer DMA descriptors while
                # keeping the drain pipelined.
                for ph in range(2):
                    o_sb = opool.tile(
                        [128, 2, S], mybir.dt.float32, tag="o", name=f"o_{i}_{ph}"
                    )
                    for pj in range(2):
                        pt = ph * 2 + pj
                        ps2 = ps2pool.tile(
                            [128, S], mybir.dt.float32, tag="ps2", name=f"ps2_{i}_{pt}"
                        )
                        for nt in range(T):
                            nc.tensor.matmul(
                                ps2[:],
                                dt_sb[:, nt, pt * 128 : (pt + 1) * 128],
                                ct_sb[:, nt, :],
                                start=(nt == 0),
                                stop=(nt == T - 1),
                            )
                        nc.scalar.copy(o_sb[:, pj, :], ps2[:])
                    if ph == 0:
                        nc.scalar.dma_start(out_v[i, :, 0:2, :], o_sb[:])
                    else:
                        nc.sync.dma_start(out_v[i, :, 2:4, :], o_sb[:])
    nc.compile()
    return nc


_NC_CACHE = None


def _get_nc():
    global _NC_CACHE
    if _NC_CACHE is None:
        _NC_CACHE = _build()
    return _NC_CACHE


def run_sharded(img: np.ndarray, **spmd_kwargs):
    """img [128, 1, 512, 512] f32 -> (out [128, 1, 512, 512] f32, BassKernelResults)."""
    img = np.ascontiguousarray(np.asarray(img, dtype=np.float32)).reshape(B_FULL, S, S)
    imga = np.ascontiguousarray(img[:, :H, :])
    imgb = np.ascontiguousarray(img[:, :H - 1 :-1, :])  # rows 511..256 reversed
    ct = _basis_f32()
    ce = np.ascontiguousarray(ct[:H, 0::2])
    co = np.ascontiguousarray(ct[:H, 1::2])
    nc = _get_nc()
    in_maps = [
        {
            "imga": imga[k * BPC : (k + 1) * BPC],
            "imgb": imgb[k * BPC : (k + 1) * BPC],
            "ct": ct,
            "ce": ce,
            "co": co,
        }
        for k in range(N_CORES)
    ]
    res = run_bass_kernel_spmd(nc, in_maps, core_ids=list(range(N_CORES)), **spmd_kwargs)
    out = np.empty((B_FULL, S, S), dtype=np.float32)
    for k in range(N_CORES):
        out[k * BPC : (k + 1) * BPC] = res.results[k]["out"]
    return out.reshape(B_FULL, 1, S, S), res


def kernel(img: np.ndarray) -> np.ndarray:
    out, _ = run_sharded(img)
    return out


# revision 28
# speedup vs baseline: 1.1390x; 1.0131x over previous
"""2D DCT-II (separable) kernel for Trainium2, data-parallel over 8 NeuronCores.

Problem: img [128, 1, 512, 512] f32 -> out [128, 1, 512, 512] f32 with
    out[b,0] = scale * (Cp @ img[b,0] @ Cq^T)
where Cp[p,m] = cos(pi*(2m+1)*p/1024), Cq[q,n] = cos(pi*(2n+1)*q/1024) and
scale[p,q] = (2/512)*row[p]*col[q] (1/sqrt2 on p==0 / q==0). Since M=N=512 the
two basis matrices are identical; the rank-1 scale is folded into them:
    C'[k,j] = s_k * cos(pi*(2j+1)*k/1024),  s_k = sqrt(2/512) * (1/sqrt2 if k==0 else 1)
    out[b] = C' @ img[b] @ C'^T

Per-core (16 images each): two PE matmul stages with the image/intermediate as
the stationary operand (both stages contract over the data's partition dim, so
no transposes are needed):
    stage1: Dt[n, p] = sum_m A[m, n] * C'T[m, p]   (lhsT = A tile, rhs = C'T)
    stage2: Y[p, q]  = sum_n Dt[n, p] * C'T[n, q]  (lhsT = Dt tile, rhs = C'T)
Matmuls run in float32r (TF32-like, ~11 mantissa bits) at full PE rate.

Stage 1 exploits the DCT-II reflection symmetry C'[p, 511-m] = (-1)^p C'[p, m]:
with E[m'] = A[m'] + A[511-m'] and O[m'] = A[m'] - A[511-m'] (m' < 256),
even output rows come from a 256-contraction with E, odd rows from O —
half the stage-1 PE cycles. The host passes the image as two row-halves
(bottom half row-reversed) so the fold pairs are partition-aligned; E/O are
formed on DVE/GpSimd; the even/odd output-row interleave happens inside the
PSUM->SBUF copies (stride-2 writes, same 1x copy cost).
"""

import sys
import numpy as np

for _p in ("/opt/trn_rl_repo", "/opt/pypackages"):
    if _p not in sys.path:
        sys.path.append(_p)

import concourse.tile as tile  # noqa: E402
from concourse import bacc, mybir  # noqa: E402
from concourse.bass_utils import run_bass_kernel_spmd  # noqa: E402

N_CORES = 8
B_FULL = 128
S = 512  # image side
H = S // 2
BPC = B_FULL // N_CORES  # images per core
T = S // 128  # 4 partition tiles per image side


def _basis_f32() -> np.ndarray:
    """C'T[j, k] = s_k * cos(pi*(2j+1)*k/1024), shape [512, 512] f32."""
    j = np.arange(S, dtype=np.float64)
    k = np.arange(S, dtype=np.float64)
    c = np.cos(np.pi * (2.0 * j[:, None] + 1.0) * k[None, :] / (2.0 * S))
    s = np.full(S, np.sqrt(2.0 / S), dtype=np.float64)
    s[0] /= np.sqrt(2.0)
    return (c * s[None, :]).astype(np.float32)


def _build():
    nc = bacc.Bacc("TRN2", target_bir_lowering=False, debug=False)
    # Image passed as two halves: rows 0:256 forward, rows 511:255 reversed
    # (host-side flip) so fold pairs m' <-> 511-m' are partition-aligned with
    # plain positive-stride DMAs.
    imga_d = nc.dram_tensor(
        "imga", [BPC, H, S], mybir.dt.float32r, kind="ExternalInput"
    ).ap()
    imgb_d = nc.dram_tensor(
        "imgb", [BPC, H, S], mybir.dt.float32r, kind="ExternalInput"
    ).ap()
    ct_d = nc.dram_tensor("ct", [S, S], mybir.dt.float32r, kind="ExternalInput").ap()
    ce_d = nc.dram_tensor("ce", [H, H], mybir.dt.float32r, kind="ExternalInput").ap()
    co_d = nc.dram_tensor("co", [H, H], mybir.dt.float32r, kind="ExternalInput").ap()
    out_d = nc.dram_tensor("out", [BPC, S, S], mybir.dt.float32, kind="ExternalOutput").ap()

    out_v = out_d.rearrange("b (t p) q -> b p t q", p=128)
    imga_v = imga_d.rearrange("b (t p) n -> b p t n", p=128)
    imgb_v = imgb_d.rearrange("b (t p) n -> b p t n", p=128)
    ct_v = ct_d.rearrange("(t p) k -> t p k", p=128)
    ce_v = ce_d.rearrange("(t p) k -> t p k", p=128)
    co_v = co_d.rearrange("(t p) k -> t p k", p=128)

    with tile.TileContext(nc) as tc:
        with (
            tc.tile_pool(name="const", bufs=1) as cpool,
            tc.tile_pool(name="a", bufs=10) as apool,
            tc.tile_pool(name="eo", bufs=16) as eopool,
            tc.tile_pool(name="dt", bufs=2) as dtpool,
            tc.tile_pool(name="o", bufs=8) as opool,
            tc.tile_pool(name="ps1", bufs=4, space="PSUM") as ps1pool,
            tc.tile_pool(name="ps2", bufs=4, space="PSUM") as ps2pool,
        ):
            # ce tile 0 first — the very first matmul needs only it plus
            # image 0's two halves; the remaining constants follow them.
            ce_sb = cpool.tile([128, 2, H], mybir.dt.float32r)
            co_sb = cpool.tile([128, 2, H], mybir.dt.float32r)
            ct_sb = cpool.tile([128, T, S], mybir.dt.float32r)
            nc.sync.dma_start(co_sb[:, 0, :], co_v[0])

            for i in range(BPC):
                # Forward half [m' tile, n] and reversed half: ar[:, t, :]
                # holds rows 511-... so partition j of (af[:,t], ar[:,t]) is
                # the fold pair (m', 511-m').
                af = apool.tile([128, 2, S], mybir.dt.float32r, tag="a", name=f"af_{i}")
                ar = apool.tile([128, 2, S], mybir.dt.float32r, tag="a", name=f"ar_{i}")
                nc.sync.dma_start(af[:], imga_v[i])
                nc.sync.dma_start(ar[:], imgb_v[i])
                if i == 0:
                    # Remaining constants, ordered by first use.
                    nc.sync.dma_start(co_sb[:, 1, :], co_v[1])
                    for t in range(2):
                        nc.sync.dma_start(ce_sb[:, t, :], ce_v[t])
                    for t in range(T):
                        nc.sync.dma_start(ct_sb[:, t, :], ct_v[t])

                # Fold: E = A + rev(A), O = A - rev(A)  (each [256, 512], 2 tiles)
                e0 = eopool.tile([128, S], mybir.dt.float32r, tag="eo", name=f"e0_{i}")
                e1 = eopool.tile([128, S], mybir.dt.float32r, tag="eo", name=f"e1_{i}")
                o0 = eopool.tile([128, S], mybir.dt.float32r, tag="eo", name=f"o0_{i}")
                o1 = eopool.tile([128, S], mybir.dt.float32r, tag="eo", name=f"o1_{i}")
                nc.gpsimd.tensor_sub(o0[:], af[:, 0, :], ar[:, 0, :])
                nc.gpsimd.tensor_sub(o1[:], af[:, 1, :], ar[:, 1, :])
                nc.vector.tensor_add(e0[:], af[:, 0, :], ar[:, 0, :])
                nc.vector.tensor_add(e1[:], af[:, 1, :], ar[:, 1, :])
                e_t, o_t = (e0, e1), (o0, o1)

                # stage 1 (folded): Dt[n, 2k] from E/ce, Dt[n, 2k+1] from O/co.
                # ps1[nt] cols 0:256 hold even-p, cols 256:512 odd-p.
                ps1 = [ps1pool.tile([128, S], mybir.dt.float32, tag="ps1", name=f"ps1_{i}_{j}") for j in range(T)]
                # O-part first: the gpsimd subs are ready earlier than the DVE
                # adds (which queue behind the previous image's mid copies).
                for nt in range(T):
                    nts = slice(nt * 128, (nt + 1) * 128)
                    for mh in range(2):
                        nc.tensor.matmul(
                            ps1[nt][:, H:S],
                            o_t[mh][:, nts],
                            co_sb[:, mh, :],
                            start=(mh == 0),
                            stop=(mh == 1),
                        )
                    for mh in range(2):
                        nc.tensor.matmul(
                            ps1[nt][:, 0:H],
                            e_t[mh][:, nts],
                            ce_sb[:, mh, :],
                            start=(mh == 0),
                            stop=(mh == 1),
                        )
                dt_sb = dtpool.tile([128, T, S], mybir.dt.float32r, tag="dt")
                for nt in range(T):
                    # One mid-copy pair on ACT to keep DVE under the PE span.
                    eng = nc.scalar.copy if nt == 3 else nc.vector.tensor_copy
                    eng(dt_sb[:, nt, 0:S:2], ps1[nt][:, 0:H])
                    eng(dt_sb[:, nt, 1:S:2], ps1[nt][:, H:S])

                # stage 2 (p-outer): Y[p, q] = sum_n Dt[n, p] C'T[n, q]
                # Output staged in 2-tile chunks: fewer DMA descriptors while
                # keeping the drain pipelined.
                for ph in range(2):
                    o_sb = opool.tile(
                        [128, 2, S], mybir.dt.float32, tag="o", name=f"o_{i}_{ph}"
                    )
                    for pj in range(2):
                        pt = ph * 2 + pj
                        ps2 = ps2pool.tile(
                            [128, S], mybir.dt.float32, tag="ps2", name=f"ps2_{i}_{pt}"
                        )
                        for nt in range(T):
                            nc.tensor.matmul(
                                ps2[:],
                                dt_sb[:, nt, pt * 128 : (pt + 1) * 128],
                                ct_sb[:, nt, :],
                                start=(nt == 0),
                                stop=(nt == T - 1),
                            )
                        nc.scalar.copy(o_sb[:, pj, :], ps2[:])
                    if ph == 0:
                        nc.scalar.dma_start(out_v[i, :, 0:2, :], o_sb[:])
                    else:
                        nc.sync.dma_start(out_v[i, :, 2:4, :], o_sb[:])
    nc.compile()
    return nc


_NC_CACHE = None


def _get_nc():
    global _NC_CACHE
    if _NC_CACHE is None:
        _NC_CACHE = _build()
    return _NC_CACHE


def run_sharded(img: np.ndarray, **spmd_kwargs):
    """img [128, 1, 512, 512] f32 -> (out [128, 1, 512, 512] f32, BassKernelResults)."""
    img = np.ascontiguousarray(np.asarray(img, dtype=np.float32)).reshape(B_FULL, S, S)
    imga = np.ascontiguousarray(img[:, :H, :])
    imgb = np.ascontiguousarray(img[:, :H - 1 :-1, :])  # rows 511..256 reversed
    ct = _basis_f32()
    ce = np.ascontiguousarray(ct[:H, 0::2])
    co = np.ascontiguousarray(ct[:H, 1::2])
    nc = _get_nc()
    in_maps = [
        {
            "imga": imga[k * BPC : (k + 1) * BPC],
            "imgb": imgb[k * BPC : (k + 1) * BPC],
            "ct": ct,
            "ce": ce,
            "co": co,
        }
        for k in range(N_CORES)
    ]
    res = run_bass_kernel_spmd(nc, in_maps, core_ids=list(range(N_CORES)), **spmd_kwargs)
    out = np.empty((B_FULL, S, S), dtype=np.float32)
    for k in range(N_CORES):
        out[k * BPC : (k + 1) * BPC] = res.results[k]["out"]
    return out.reshape(B_FULL, 1, S, S), res


def kernel(img: np.ndarray) -> np.ndarray:
    out, _ = run_sharded(img)
    return out
